# revision 1
# baseline (speedup 1.0000x reference)
"""Calibrated cross-entropy 2D (histogram binning) — Trainium2 Bass kernel.

Problem: nn_CalibratedCE2d_88493506167215
  predict    [8, 21, 513, 513] f32   (NCHW logits)
  target     [8, 513, 513]     int   (class ids)
  confidence [2105352]         f32
  accuracies [15]              f32
  n_bin      15

  loss = -sum_i w_i * logp_target_i / size
  where w_i = coeff[bin(confidence_i)] if selected else 0,
        coeff_b = acc_b*10 - (1-acc_b)*50 (only coeff>0 bins selected),
        size = number of selected pixels.

Sharding: data-parallel over the batch axis — one image (n) per NeuronCore,
8 cores.  Per-core device program (pixel-major [128, F] tiles):
  for each class c in 0..20:
      load plane slice x_c, e_c = exp(x_c)                 (ACT, bf16 out)
      masked_c = (tgt == c) * e_c                          (DVE fused stt)
      PSUM A += I @ e_c ; PSUM B += I @ masked_c           (PE identity matmuls)
  A = sum_c exp(x_c) per pixel, B = exp(x_target) per pixel
  logp_t = ln(B) - ln(A)
  out partials: sum_f w*ln(B), sum_f w*ln(A)               (DVE stt + accum)
Host: per-pixel weights w from confidence (identical f32 arithmetic as the
reference), 8-way partial-sum combine, final divide.  The last pixel of each
image (263169 = 128*2056 + 1 does not tile evenly) is folded in on the host.
"""

import numpy as np
import ml_dtypes
from contextlib import ExitStack

N_IMG, C, H, W = 8, 21, 513, 513
PX = H * W                    # 263169 pixels per image
FD = 2056                     # tile free dim (2048 main grid + 8 tail cols)
MFD = 2048                    # main grid columns -> PSUM chains (4 banks each)
MAIN = 128 * MFD              # 262144 pixels in the main grid
LEFT = MAIN + 128 * 8         # 263168; the final pixel is handled on the host
N_TOTAL_BINS = 15

_NC_CACHE: dict = {}


def _build_program():
    import concourse.bass as bass
    import concourse.bacc as bacc
    import concourse.tile as tile
    from concourse import mybir

    f32 = mybir.dt.float32
    bf16 = mybir.dt.bfloat16
    Exp = mybir.ActivationFunctionType.Exp
    Ln = mybir.ActivationFunctionType.Ln
    is_equal = mybir.AluOpType.is_equal
    mult = mybir.AluOpType.mult
    bypass = mybir.AluOpType.bypass

    nc = bacc.Bacc(
        "TRN2",
        target_bir_lowering=False,
        debug=False,
        enable_asserts=False,
        num_devices=N_IMG,
    )
    x_d = nc.dram_tensor("x", [C, PX], f32, kind="ExternalInput")
    tgt_d = nc.dram_tensor("tgt", [PX], bf16, kind="ExternalInput")
    w_d = nc.dram_tensor("w", [PX], f32, kind="ExternalInput")
    id_d = nc.dram_tensor("ident", [128, 128], bf16, kind="ExternalInput")
    # host-packed tail sidecar: pixels MAIN..LEFT as [128, 21*8] / [128, 8]
    xt_d = nc.dram_tensor("xt", [128, C * 8], f32, kind="ExternalInput")
    mkt_d = nc.dram_tensor("mkt", [128, C * 8], bf16, kind="ExternalInput")
    w8_d = nc.dram_tensor("w8", [128, 8], f32, kind="ExternalInput")
    out_d = nc.dram_tensor("out", [128, 10], f32, kind="ExternalOutput")

    x = x_d.ap()
    tgt = tgt_d.ap()
    w = w_d.ap()

    with tile.TileContext(nc) as tc, ExitStack() as ctx:
        const_pool = ctx.enter_context(tc.tile_pool(name="const", bufs=1))
        xpool = ctx.enter_context(tc.tile_pool(name="xp", bufs=8))
        epool = ctx.enter_context(tc.tile_pool(name="ep", bufs=8))
        kpool = ctx.enter_context(tc.tile_pool(name="kp", bufs=3))
        mpool = ctx.enter_context(tc.tile_pool(name="mp", bufs=8))
        postpool = ctx.enter_context(tc.tile_pool(name="post", bufs=1))
        psum = ctx.enter_context(tc.tile_pool(name="ps", bufs=1, space="PSUM"))

        zb = const_pool.tile([128, 1], f32, tag="zb", name="zb")
        nc.vector.memset(zb[:], 0.0)
        ob = const_pool.tile([128, 1], f32, tag="ob", name="ob")
        nc.vector.memset(ob[:], 1.0)
        # dummy activations: hoist the ACT table loads to kernel start so
        # they overlap the DMA ramp instead of gating the first/last real op
        dum = const_pool.tile([128, 2], f32, tag="dum", name="dum")
        nc.scalar.activation(dum[:, 0:1], zb[:], Ln, bias=ob[:, 0:1])
        nc.scalar.activation(dum[:, 1:2], zb[:], Exp, bias=zb[:, 0:1])

        tgt_m = const_pool.tile([128, MFD], bf16, tag="tgtm", name="tgt_m")
        w_m = const_pool.tile([128, MFD], f32, tag="wm", name="w_m")
        idt = const_pool.tile([128, 128], bf16, tag="idt", name="idt")
        xt = const_pool.tile([128, C * 8], f32, tag="xt", name="xt")
        mkt = const_pool.tile([128, C * 8], bf16, tag="mkt", name="mkt")
        w8 = const_pool.tile([128, 8], f32, tag="w8", name="w8")

        # A = sum_c exp(x_c), B = exp(x_target): PE psum chains over the main
        # 2048 columns; the 1024-px tail sidecar reduces on DVE.
        A = psum.tile([128, MFD], f32, tag="A", name="A")
        B = psum.tile([128, MFD], f32, tag="B", name="B")

        def load_x(c):
            t = xpool.tile([128, MFD], f32, tag="xm", name=f"xm{c}")
            nc.sync.dma_start(
                t[:], x[c : c + 1, 0:MAIN].rearrange("o (p f) -> (o p) f", p=128)
            )
            return t

        acc = postpool.tile([128, 10], f32, tag="acc", name="acc")
        nc.vector.memset(acc[:], 0.0)

        def emit_tail_sidecar():
            # 1024-px tail: one exp + mask-mul + class-axis reduces + post.
            # No dependency on the psum chains — emitted mid-loop so it
            # fills DMA-wait bubbles instead of serializing at the end.
            et_all = const_pool.tile([128, C * 8], bf16, tag="eta", name="et_all")
            nc.scalar.activation(et_all[:], xt[:], Exp, bias=zb[:, 0:1])
            mt_all = const_pool.tile([128, C * 8], bf16, tag="mta", name="mt_all")
            nc.vector.tensor_tensor(mt_all[:], mkt[:], et_all[:], op=mult)
            At = const_pool.tile([128, 8], f32, tag="At", name="At")
            Bt = const_pool.tile([128, 8], f32, tag="Bt", name="Bt")
            nc.vector.tensor_reduce(
                At[:], et_all[:].rearrange("p (c j) -> p j c", c=C),
                axis=mybir.AxisListType.X, op=mybir.AluOpType.add,
            )
            nc.vector.tensor_reduce(
                Bt[:], mt_all[:].rearrange("p (c j) -> p j c", c=C),
                axis=mybir.AxisListType.X, op=mybir.AluOpType.add,
            )
            lbt = const_pool.tile([128, 8], f32, tag="lbt", name="lbt")
            lat = const_pool.tile([128, 8], f32, tag="lat", name="lat")
            scrt = const_pool.tile([128, 8], f32, tag="scrt", name="scrt")
            nc.scalar.activation(lbt[:], Bt[:], Ln, bias=zb[:, 0:1])
            nc.scalar.activation(lat[:], At[:], Ln, bias=zb[:, 0:1])
            nc.vector.scalar_tensor_tensor(
                scrt[:], lbt[:], 0.0, w8[:],
                op0=bypass, op1=mult, accum_out=acc[:, 8:9],
            )
            nc.vector.scalar_tensor_tensor(
                scrt[:], lat[:], 0.0, w8[:],
                op0=bypass, op1=mult, accum_out=acc[:, 9:10],
            )

        xms = {0: load_x(0)}
        for c in range(C):
            xm = xms.pop(c)
            if c == 0:
                nc.sync.dma_start(
                    tgt_m[:], tgt[0:MAIN].rearrange("(p f) -> p f", p=128)
                )
                nc.sync.dma_start(idt[:], id_d.ap())
            if c + 1 < C:
                xms[c + 1] = load_x(c + 1)
            if c == 2:
                nc.sync.dma_start(xt[:], xt_d.ap())
                nc.sync.dma_start(mkt[:], mkt_d.ap())
                nc.sync.dma_start(w8[:], w8_d.ap())
            if c == 4:
                nc.sync.dma_start(
                    w_m[:], w[0:MAIN].rearrange("(p f) -> p f", p=128)
                )
            em = epool.tile([128, MFD], bf16, tag="em", name=f"em{c}")
            nc.scalar.activation(em[:], xm[:], Exp, bias=zb[:, 0:1])
            mk = kpool.tile([128, MFD], bf16, tag="mk", name=f"mk{c}")
            nc.vector.tensor_scalar(mk[:], tgt_m[:], float(c), None, op0=is_equal)
            mm = mpool.tile([128, MFD], bf16, tag="mm", name=f"mm{c}")
            nc.vector.tensor_tensor(mm[:], mk[:], em[:], op=mult)
            for j in range(MFD // 512):
                sl = slice(j * 512, (j + 1) * 512)
                nc.tensor.matmul(
                    A[:, sl], idt[:], em[:, sl], start=(c == 0), stop=(c == C - 1)
                )
                nc.tensor.matmul(
                    B[:, sl], idt[:], mm[:, sl], start=(c == 0), stop=(c == C - 1)
                )
            if c == 5:
                emit_tail_sidecar()

        # ---- post: logp_t = ln(B) - ln(A); accumulate w-weighted sums.
        # Column halves pipeline ACT(ln) with DVE(weighted reduce).
        lb = postpool.tile([128, MFD], f32, tag="lb", name="lb")
        la = postpool.tile([128, MFD], f32, tag="la", name="la")
        scr = postpool.tile([128, MFD], f32, tag="scr", name="scr")
        HH = MFD // 2
        for h in range(2):
            sl = slice(h * HH, (h + 1) * HH)
            nc.scalar.activation(lb[:, sl], B[:, sl], Ln, bias=zb[:, 0:1])
            nc.vector.scalar_tensor_tensor(
                scr[:, sl], lb[:, sl], 0.0, w_m[:, sl],
                op0=bypass, op1=mult, accum_out=acc[:, 4 * h : 4 * h + 1],
            )
            nc.scalar.activation(la[:, sl], A[:, sl], Ln, bias=zb[:, 0:1])
            nc.vector.scalar_tensor_tensor(
                scr[:, sl], la[:, sl], 0.0, w_m[:, sl],
                op0=bypass, op1=mult, accum_out=acc[:, 4 * h + 1 : 4 * h + 2],
            )
        nc.sync.dma_start(out_d.ap(), acc[:])

    nc.compile()
    return nc


def _get_nc():
    if "nc" not in _NC_CACHE:
        _NC_CACHE["nc"] = _build_program()
    return _NC_CACHE["nc"]


def _pixel_weights(conf: np.ndarray, accuracies: np.ndarray, n_bin: int):
    """Per-pixel weights, f32 arithmetic identical to the reference."""
    acc = np.asarray(accuracies, dtype=np.float32)[:n_bin]
    coeff = acc * np.float32(10.0) - (np.float32(1.0) - acc) * np.float32(50.0)
    wtab = np.where(coeff > np.float32(0.0), coeff, np.float32(0.0)).astype(np.float32)
    # table16[k] for k = ceil(conf*15) in 0..15; k=0 (conf==0) -> invalid -> 0
    table16 = np.concatenate([[np.float32(0.0)], wtab]).astype(np.float32)
    t15 = conf * np.float32(N_TOTAL_BINS)          # same f32 product as reference
    k16 = np.ceil(t15).astype(np.int32)
    k16 = np.clip(k16, 0, n_bin)
    wfull = table16[k16]
    valid = (conf > np.float32(0.0)) & (conf <= np.float32(1.0))
    wfull = np.where(valid, wfull, np.float32(0.0)).astype(np.float32)
    return wfull


def _prepare(predict, target, confidence, accuracies, n_bin):
    predict = np.ascontiguousarray(np.asarray(predict, dtype=np.float32))
    target = np.asarray(target)
    conf = np.asarray(confidence, dtype=np.float32)
    accuracies = np.asarray(accuracies, dtype=np.float32)
    n_bin = int(n_bin)
    assert predict.shape == (N_IMG, C, H, W) and n_bin == N_TOTAL_BINS

    wfull = _pixel_weights(conf, accuracies, n_bin)
    size = float(np.count_nonzero(wfull))

    xs = predict.reshape(N_IMG, C, PX)
    tg = target.reshape(N_IMG, PX).astype(np.int64)
    wf = wfull.reshape(N_IMG, PX)
    ident = np.eye(128, dtype=ml_dtypes.bfloat16)

    in_maps = []
    for n in range(N_IMG):
        # tail sidecar: pixels MAIN..LEFT as [128, 8], classes side by side
        xt = np.ascontiguousarray(
            xs[n][:, MAIN:LEFT].reshape(C, 128, 8).transpose(1, 0, 2).reshape(128, C * 8)
        )
        tail_t = tg[n][MAIN:LEFT].reshape(128, 8)
        onehot = (tail_t[None, :, :] == np.arange(C)[:, None, None])
        mkt = np.ascontiguousarray(
            onehot.transpose(1, 0, 2).reshape(128, C * 8)
        ).astype(ml_dtypes.bfloat16)
        w8 = np.ascontiguousarray(wf[n][MAIN:LEFT].reshape(128, 8))
        in_maps.append(
            {
                "x": xs[n],
                "tgt": tg[n].astype(ml_dtypes.bfloat16),
                "w": wf[n],
                "ident": ident,
                "xt": xt,
                "mkt": mkt,
                "w8": w8,
            }
        )
    return xs, tg, wf, size, in_maps


def _combine(res_list, xs, tg, wf, size) -> np.ndarray:
    S = 0.0
    for n in range(N_IMG):
        o = np.asarray(res_list[n]["out"], dtype=np.float64)
        # written accumulator columns: (w*lnB, w*lnA) pairs at 0,4,8
        S += sum(o[:, j].sum() - o[:, j + 1].sum() for j in (0, 4, 8))

    # host-side leftover pixels (one per image: index LEFT..PX-1)
    for n in range(N_IMG):
        for p in range(LEFT, PX):
            xv = xs[n][:, p].astype(np.float64)
            m = xv.max()
            lse = np.log(np.exp(xv - m).sum()) + m
            xt = xv[tg[n][p]]
            S += float(wf[n][p]) * (xt - lse)

    loss = np.float32(-(S / size))
    return np.asarray(loss, dtype=np.float32)


def run_device(in_maps, trace=False, **kwargs):
    from concourse.bass_utils import run_bass_kernel_spmd

    nc = _get_nc()
    return run_bass_kernel_spmd(
        nc, in_maps, core_ids=list(range(N_IMG)), trace=trace, **kwargs
    )


def kernel(predict, target, confidence, accuracies, n_bin) -> np.ndarray:
    xs, tg, wf, size, in_maps = _prepare(predict, target, confidence, accuracies, n_bin)
    res = run_device(in_maps)
    return _combine(res.results, xs, tg, wf, size)



# revision 2
# speedup vs baseline: 2.4115x; 2.4115x over previous
"""Calibrated cross-entropy 2D (histogram binning) — Trainium2 Bass kernel.

Problem: nn_CalibratedCE2d_88493506167215
  predict    [8, 21, 513, 513] f32   (NCHW logits)
  target     [8, 513, 513]     int   (class ids)
  confidence [2105352]         f32
  accuracies [15]              f32
  n_bin      15

  loss = -sum_i w_i * logp_target_i / size
  where w_i = coeff[bin(confidence_i)] if selected else 0,
        coeff_b = acc_b*10 - (1-acc_b)*50 (only coeff>0 bins selected),
        size = number of selected pixels.

Key structure: only pixels in positive-coefficient bins contribute (for this
regime ~20% of pixels).  The host computes the per-pixel weights (identical
f32 arithmetic to the reference — this is the same binning prep the previous
version did), compacts the selected pixel columns, and shards them evenly
across the 8 NeuronCores.  Each core's device program does the heavy math:

  for each pixel group g:   (pipelined: DMA || ACT || DVE)
      load x_g  [128, 21*Fg] bf16     (classes side by side per partition)
      e_g = exp(x_g)                  (ACT, the only transcendental on device)
      A_g[p,f] = sum_c e_g[p,c,f]     (DVE tensor_reduce over class axis)
      store A_g [128, Fg] f32

A is the per-pixel sum of exponentials; the host finishes with
S = sum w*(x_t - ln A) in f64 (8-way partial combine = the all-reduce),
loss = -S/size.  x_t (the target logit) is an exact gather, done host-side
with the same fancy indexing that builds the compacted input.
"""

import math

import numpy as np
import ml_dtypes

N_IMG, C, H, W = 8, 21, 513, 513
PX = H * W                     # 263169 pixels per image
NPIX = N_IMG * PX              # 2105352 total
N_CORES = 8
N_TOTAL_BINS = 15

_NC_CACHE: dict = {}


def _group_plan(F: int) -> list:
    """Split the per-core F pixel-columns into pipeline groups.

    Small first group (fast pipeline fill), ~112-col steady-state groups,
    small last group (short DVE/out tail)."""
    if F <= 72:
        return [F]
    sizes = [48]
    rem = F - 48
    while rem > 136:
        sizes.append(112)
        rem -= 112
    if rem > 48:
        sizes.append(rem - 24)
        sizes.append(24)
    else:
        sizes.append(rem)
    return sizes


def _build_program(sizes):
    import concourse.bass as bass  # noqa: F401
    import concourse.bacc as bacc
    import concourse.tile as tile
    from concourse import mybir
    from contextlib import ExitStack

    f32 = mybir.dt.float32
    bf16 = mybir.dt.bfloat16
    Exp = mybir.ActivationFunctionType.Exp

    F = sum(sizes)
    G = len(sizes)
    offs = [0]
    for s in sizes:
        offs.append(offs[-1] + s)

    nc = bacc.Bacc(
        "TRN2",
        target_bir_lowering=False,
        debug=False,
        enable_asserts=False,
        num_devices=N_CORES,
    )
    x_d = nc.dram_tensor("x", [128, C * F], bf16, kind="ExternalInput")
    out_d = nc.dram_tensor("out", [128, F], f32, kind="ExternalOutput")
    xap = x_d.ap()
    oap = out_d.ap()

    with tile.TileContext(nc) as tc, ExitStack() as ctx:
        const_pool = ctx.enter_context(tc.tile_pool(name="const", bufs=1))
        xpool = ctx.enter_context(tc.tile_pool(name="xp", bufs=3))
        epool = ctx.enter_context(tc.tile_pool(name="ep", bufs=2))
        apool = ctx.enter_context(tc.tile_pool(name="ap", bufs=2))

        zb = const_pool.tile([128, 1], f32, tag="zb", name="zb")
        nc.vector.memset(zb[:], 0.0)
        # dummy exp: hoists the ACT table load to kernel start so the
        # ~2.7us load overlaps the first DMA instead of gating it
        dum = const_pool.tile([128, 1], f32, tag="dum", name="dum")
        nc.scalar.activation(dum[:], zb[:], Exp, bias=zb[:, 0:1])

        def load_group(g):
            s = sizes[g]
            t = xpool.tile([128, C * s], bf16, tag="xg", name=f"xg{g}")
            nc.sync.dma_start(t[:], xap[:, C * offs[g] : C * offs[g + 1]])
            return t

        xg = {0: load_group(0)}
        if G > 1:
            xg[1] = load_group(1)
        for g in range(G):
            s = sizes[g]
            xm = xg.pop(g)
            if g + 2 < G:
                xg[g + 2] = load_group(g + 2)
            em = epool.tile([128, C * s], bf16, tag="eg", name=f"eg{g}")
            nc.scalar.activation(em[:], xm[:], Exp, bias=zb[:, 0:1])
            A = apool.tile([128, s], f32, tag="ag", name=f"ag{g}")
            nc.vector.tensor_reduce(
                A[:],
                em[:].rearrange("p (c f) -> p f c", c=C),
                axis=mybir.AxisListType.X,
                op=mybir.AluOpType.add,
            )
            nc.sync.dma_start(oap[:, offs[g] : offs[g + 1]], A[:])

    nc.compile()
    return nc


def _get_nc(sizes):
    key = tuple(sizes)
    if key not in _NC_CACHE:
        _NC_CACHE[key] = _build_program(sizes)
    return _NC_CACHE[key]


def _pixel_weights(conf: np.ndarray, accuracies: np.ndarray, n_bin: int):
    """Per-pixel weights, f32 arithmetic identical to the reference."""
    acc = np.asarray(accuracies, dtype=np.float32)[:n_bin]
    coeff = acc * np.float32(10.0) - (np.float32(1.0) - acc) * np.float32(50.0)
    wtab = np.where(coeff > np.float32(0.0), coeff, np.float32(0.0)).astype(np.float32)
    # table16[k] for k = ceil(conf*15) in 0..15; k=0 (conf==0) -> invalid -> 0
    table16 = np.concatenate([[np.float32(0.0)], wtab]).astype(np.float32)
    t15 = conf * np.float32(N_TOTAL_BINS)          # same f32 product as reference
    k16 = np.ceil(t15).astype(np.int32)
    k16 = np.clip(k16, 0, n_bin)
    wfull = table16[k16]
    valid = (conf > np.float32(0.0)) & (conf <= np.float32(1.0))
    wfull = np.where(valid, wfull, np.float32(0.0)).astype(np.float32)
    return wfull


def _prepare(predict, target, confidence, accuracies, n_bin):
    predict = np.ascontiguousarray(np.asarray(predict, dtype=np.float32))
    target = np.asarray(target)
    conf = np.asarray(confidence, dtype=np.float32)
    accuracies = np.asarray(accuracies, dtype=np.float32)
    n_bin = int(n_bin)
    assert predict.shape == (N_IMG, C, H, W) and n_bin == N_TOTAL_BINS

    wfull = _pixel_weights(conf, accuracies, n_bin)
    size = float(np.count_nonzero(wfull))
    idx = np.flatnonzero(wfull)
    nsel = int(idx.size)
    if nsel == 0:
        return None, None, size, None, None

    F = max(1, math.ceil(nsel / (N_CORES * 128)))
    sizes = _group_plan(F)
    P = 128 * F
    T = N_CORES * P

    xs = predict.reshape(N_IMG, C, PX)
    tgt = target.reshape(-1).astype(np.int64)

    # compacted logits for the selected pixels: XL [C, nsel] (f32)
    XL = np.empty((C, nsel), np.float32)
    bounds = np.searchsorted(idx, np.arange(N_IMG + 1) * PX)
    for n in range(N_IMG):
        lo, hi = bounds[n], bounds[n + 1]
        if hi > lo:
            XL[:, lo:hi] = xs[n][:, idx[lo:hi] - n * PX]

    # exact host-side pieces: target logit gather + weights
    tsel = tgt[idx]
    xt = XL[tsel, np.arange(nsel)].astype(np.float64)
    wsel = wfull[idx].astype(np.float64)
    S1 = float(np.dot(wsel, xt))

    # pad to the 8-core grid and pack per core with group-major layout:
    # core k, group g block = [128, C, Fg]  (class-minor within the block)
    XLb = XL.astype(ml_dtypes.bfloat16)
    if T > nsel:
        XLb = np.concatenate(
            [XLb, np.zeros((C, T - nsel), ml_dtypes.bfloat16)], axis=1
        )
    offs = np.concatenate([[0], np.cumsum(sizes)])
    in_maps = []
    for k in range(N_CORES):
        blk = XLb[:, k * P : (k + 1) * P].reshape(C, 128, F)
        pieces = [
            np.ascontiguousarray(
                blk[:, :, offs[g] : offs[g + 1]].transpose(1, 0, 2)
            ).reshape(128, C * int(sizes[g]))
            for g in range(len(sizes))
        ]
        xk = np.ascontiguousarray(np.concatenate(pieces, axis=1))
        in_maps.append({"x": xk})
    return sizes, in_maps, size, (wsel, S1, nsel), F


def _combine(res_list, host_data, size) -> np.ndarray:
    wsel, S1, nsel = host_data
    A = np.concatenate(
        [np.asarray(r["out"], dtype=np.float64).reshape(-1) for r in res_list]
    )[:nsel]
    S2 = float(np.dot(wsel, np.log(A)))
    loss = np.float32(-((S1 - S2) / size))
    return np.asarray(loss, dtype=np.float32)


def run_device(sizes, in_maps, trace=False, **kwargs):
    from concourse.bass_utils import run_bass_kernel_spmd

    nc = _get_nc(sizes)
    return run_bass_kernel_spmd(
        nc, in_maps, core_ids=list(range(N_CORES)), trace=trace, **kwargs
    )


def kernel(predict, target, confidence, accuracies, n_bin) -> np.ndarray:
    sizes, in_maps, size, host_data, F = _prepare(
        predict, target, confidence, accuracies, n_bin
    )
    if in_maps is None:
        # no selected pixels: reference computes -0/0
        return np.asarray(np.float32(np.nan))
    res = run_device(sizes, in_maps)
    return _combine(res.results, host_data, size)


# revision 4
# speedup vs baseline: 3.0971x; 1.2843x over previous
"""Calibrated cross-entropy 2D (histogram binning) — Trainium2 Bass kernel.

Problem: nn_CalibratedCE2d_88493506167215
  predict    [8, 21, 513, 513] f32   (NCHW logits)
  target     [8, 513, 513]     int   (class ids)
  confidence [2105352]         f32
  accuracies [15]              f32
  n_bin      15

  loss = -sum_i w_i * logp_target_i / size
  where w_i = coeff[bin(confidence_i)] if selected else 0,
        coeff_b = acc_b*10 - (1-acc_b)*50 (only coeff>0 bins selected),
        size = number of selected pixels.

Key structure: only pixels in positive-coefficient bins contribute (for this
regime ~20% of pixels).  The host computes the per-pixel weights (identical
f32 arithmetic to the reference — this is the same binning prep the previous
version did), compacts the selected pixel columns, and shards them evenly
across the 8 NeuronCores.  Each core's device program does the heavy math:

  for each pixel group g:   (pipelined: DMA || ACT || DVE)
      load x_g  [128, 21*Fg] bf16     (classes side by side per partition)
      e_g = exp(x_g)                  (ACT, the only transcendental on device)
      A_g[p,f] = sum_c e_g[p,c,f]     (DVE tensor_reduce over class axis)
      store A_g [128, Fg] f32

A is the per-pixel sum of exponentials; the host finishes with
S = sum w*(x_t - ln A) in f64 (8-way partial combine = the all-reduce),
loss = -S/size.  x_t (the target logit) is an exact gather, done host-side
with the same fancy indexing that builds the compacted input.
"""

import math

import numpy as np
import ml_dtypes

N_IMG, C, H, W = 8, 21, 513, 513
PX = H * W                     # 263169 pixels per image
NPIX = N_IMG * PX              # 2105352 total
N_CORES = 8
N_TOTAL_BINS = 15

_NC_CACHE: dict = {}


def _group_plan(F: int) -> list:
    """Split the per-core F pixel-columns into pipeline groups.

    Small first group (fast pipeline fill), ~112-col steady-state groups,
    small last group (short DVE/out tail)."""
    if F <= 72:
        return [F]
    sizes = [48]
    rem = F - 48
    while rem > 136:
        sizes.append(112)
        rem -= 112
    if rem > 48:
        sizes.append(rem - 24)
        sizes.append(24)
    else:
        sizes.append(rem)
    return sizes


def _build_program(sizes):
    import concourse.bass as bass  # noqa: F401
    import concourse.bacc as bacc
    import concourse.tile as tile
    from concourse import mybir
    from contextlib import ExitStack

    f32 = mybir.dt.float32
    bf16 = mybir.dt.bfloat16
    Exp = mybir.ActivationFunctionType.Exp

    F = sum(sizes)
    G = len(sizes)
    offs = [0]
    for s in sizes:
        offs.append(offs[-1] + s)

    nc = bacc.Bacc(
        "TRN2",
        target_bir_lowering=False,
        debug=False,
        enable_asserts=False,
        num_devices=N_CORES,
    )
    x_d = nc.dram_tensor("x", [128, C * F], bf16, kind="ExternalInput")
    out_d = nc.dram_tensor("out", [128, F], f32, kind="ExternalOutput")
    xap = x_d.ap()
    oap = out_d.ap()

    with tile.TileContext(nc) as tc, ExitStack() as ctx:
        const_pool = ctx.enter_context(tc.tile_pool(name="const", bufs=1))
        xpool = ctx.enter_context(tc.tile_pool(name="xp", bufs=3))
        epool = ctx.enter_context(tc.tile_pool(name="ep", bufs=2))
        apool = ctx.enter_context(tc.tile_pool(name="ap", bufs=2))

        zb = const_pool.tile([128, 1], f32, tag="zb", name="zb")
        nc.vector.memset(zb[:], 0.0)
        # dummy exp: hoists the ACT table load to kernel start so the
        # ~2.7us load overlaps the first DMA instead of gating it
        dum = const_pool.tile([128, 1], f32, tag="dum", name="dum")
        nc.scalar.activation(dum[:], zb[:], Exp, bias=zb[:, 0:1])

        def load_group(g):
            s = sizes[g]
            t = xpool.tile([128, C * s], bf16, tag="xg", name=f"xg{g}")
            nc.sync.dma_start(t[:], xap[:, C * offs[g] : C * offs[g + 1]])
            return t

        xg = {0: load_group(0)}
        if G > 1:
            xg[1] = load_group(1)
        for g in range(G):
            s = sizes[g]
            xm = xg.pop(g)
            if g + 2 < G:
                xg[g + 2] = load_group(g + 2)
            em = epool.tile([128, C * s], bf16, tag="eg", name=f"eg{g}")
            nc.scalar.activation(em[:], xm[:], Exp, bias=zb[:, 0:1])
            A = apool.tile([128, s], f32, tag="ag", name=f"ag{g}")
            # class axis is innermost (contiguous) so the reduce streams
            # at full DVE rate instead of paying for strided reads
            nc.vector.tensor_reduce(
                A[:],
                em[:].rearrange("p (f c) -> p f c", c=C),
                axis=mybir.AxisListType.X,
                op=mybir.AluOpType.add,
            )
            nc.sync.dma_start(oap[:, offs[g] : offs[g + 1]], A[:])

    nc.compile()
    return nc


def _get_nc(sizes):
    key = tuple(sizes)
    if key not in _NC_CACHE:
        _NC_CACHE[key] = _build_program(sizes)
    return _NC_CACHE[key]


def _pixel_weights(conf: np.ndarray, accuracies: np.ndarray, n_bin: int):
    """Per-pixel weights, f32 arithmetic identical to the reference."""
    acc = np.asarray(accuracies, dtype=np.float32)[:n_bin]
    coeff = acc * np.float32(10.0) - (np.float32(1.0) - acc) * np.float32(50.0)
    wtab = np.where(coeff > np.float32(0.0), coeff, np.float32(0.0)).astype(np.float32)
    # table16[k] for k = ceil(conf*15) in 0..15; k=0 (conf==0) -> invalid -> 0
    table16 = np.concatenate([[np.float32(0.0)], wtab]).astype(np.float32)
    t15 = conf * np.float32(N_TOTAL_BINS)          # same f32 product as reference
    k16 = np.ceil(t15).astype(np.int32)
    k16 = np.clip(k16, 0, n_bin)
    wfull = table16[k16]
    valid = (conf > np.float32(0.0)) & (conf <= np.float32(1.0))
    wfull = np.where(valid, wfull, np.float32(0.0)).astype(np.float32)
    return wfull


def _prepare(predict, target, confidence, accuracies, n_bin):
    predict = np.ascontiguousarray(np.asarray(predict, dtype=np.float32))
    target = np.asarray(target)
    conf = np.asarray(confidence, dtype=np.float32)
    accuracies = np.asarray(accuracies, dtype=np.float32)
    n_bin = int(n_bin)
    assert predict.shape == (N_IMG, C, H, W) and n_bin == N_TOTAL_BINS

    wfull = _pixel_weights(conf, accuracies, n_bin)
    size = float(np.count_nonzero(wfull))
    idx = np.flatnonzero(wfull)
    nsel = int(idx.size)
    if nsel == 0:
        return None, None, size, None, None

    F = max(1, math.ceil(nsel / (N_CORES * 128)))
    sizes = _group_plan(F)
    P = 128 * F
    T = N_CORES * P

    xs = predict.reshape(N_IMG, C, PX)
    tgt = target.reshape(-1).astype(np.int64)

    # compacted logits for the selected pixels: XL [C, nsel] (f32)
    XL = np.empty((C, nsel), np.float32)
    bounds = np.searchsorted(idx, np.arange(N_IMG + 1) * PX)
    for n in range(N_IMG):
        lo, hi = bounds[n], bounds[n + 1]
        if hi > lo:
            XL[:, lo:hi] = xs[n][:, idx[lo:hi] - n * PX]

    # exact host-side pieces: target logit gather + weights
    tsel = tgt[idx]
    xt = XL[tsel, np.arange(nsel)].astype(np.float64)
    wsel = wfull[idx].astype(np.float64)
    S1 = float(np.dot(wsel, xt))

    # pad to the 8-core grid and pack per core with group-major layout:
    # core k, group g block = [128, Fg, C]  (class axis contiguous)
    XLb = XL.astype(ml_dtypes.bfloat16)
    if T > nsel:
        XLb = np.concatenate(
            [XLb, np.zeros((C, T - nsel), ml_dtypes.bfloat16)], axis=1
        )
    offs = np.concatenate([[0], np.cumsum(sizes)])
    in_maps = []
    for k in range(N_CORES):
        blk = XLb[:, k * P : (k + 1) * P].reshape(C, 128, F)
        pieces = [
            np.ascontiguousarray(
                blk[:, :, offs[g] : offs[g + 1]].transpose(1, 2, 0)
            ).reshape(128, C * int(sizes[g]))
            for g in range(len(sizes))
        ]
        xk = np.ascontiguousarray(np.concatenate(pieces, axis=1))
        in_maps.append({"x": xk})
    return sizes, in_maps, size, (wsel, S1, nsel), F


def _combine(res_list, host_data, size) -> np.ndarray:
    wsel, S1, nsel = host_data
    A = np.concatenate(
        [np.asarray(r["out"], dtype=np.float64).reshape(-1) for r in res_list]
    )[:nsel]
    S2 = float(np.dot(wsel, np.log(A)))
    loss = np.float32(-((S1 - S2) / size))
    return np.asarray(loss, dtype=np.float32)


def run_device(sizes, in_maps, trace=False, **kwargs):
    from concourse.bass_utils import run_bass_kernel_spmd

    nc = _get_nc(sizes)
    return run_bass_kernel_spmd(
        nc, in_maps, core_ids=list(range(N_CORES)), trace=trace, **kwargs
    )


def kernel(predict, target, confidence, accuracies, n_bin) -> np.ndarray:
    sizes, in_maps, size, host_data, F = _prepare(
        predict, target, confidence, accuracies, n_bin
    )
    if in_maps is None:
        # no selected pixels: reference computes -0/0
        return np.asarray(np.float32(np.nan))
    res = run_device(sizes, in_maps)
    return _combine(res.results, host_data, size)
